# revision 1
# baseline (speedup 1.0000x reference)
"""Trainium2 Bass kernel for a gated cross-attention block with a dense
per-(b,h) attention bias (B=2, Q=K=2048, C=256, H=8, CH=32).

Sharding: the 16 (batch, 2-head group) units are data-parallel across 8
cores: core i handles batch b = i//4 and heads h0 = 2*(i%4), h0+1.  Linear
weights are column-sliced per head group; the output projection is
tensor-parallel over H*CH, so each core emits a partial [Q, C] output and the
host sums the 4 partials per batch (b_o is injected on exactly one core per
batch by passing zeros to the others).

Per-core dataflow (fp32 accumulate, float32r operands for full PE rate):
  - PE-transpose q_x/kv_x -> xT, project to qT/kT (c-on-partition, with an
    extra ones/mask row so Q@K^T + mask comes out of one contraction),
    gT (sigmoid with per-partition b_g bias), v (natural, with a ones column
    appended so the softmax denominator falls out of the AV matmul).
  - scores^T tiles [128 k, 512 q]: PE-transposes of the bias tile land in
    PSUM, then the QK^T matmul accumulates on top (start=False) -- the bias
    add costs zero vector-engine time.
  - exp on ScalarE reads PSUM, writes attn^T to SBUF (rounded to f32r).
  - AV matmul contracts k per 128-chunk with v_aug stationary; row 32 of the
    output is the softmax denominator.  Reciprocal (DVE) + partition
    broadcast (GPSIMD) + two vector multiplies produce the gated, normalized
    og^T.  The two heads' QK matmuls are quadrant-packed via PE row groups
    0/64 and run concurrently.
  - Final matmul og^T.T @ w_o + b_o (pre-broadcast, added on DVE) -> partial
    out, written back per 512-row q block, pipelined with the next block.
"""

import math

import numpy as np

B, Q, K, C, H, CH = 2, 2048, 2048, 256, 8, 32
N_CORES = 8
HPC = 2            # heads per core
GROUPS = H // HPC  # head groups per batch = 4

_cache = {}


def _build_nc(q=Q, k=K, tmode="f32r"):
    """Emit the per-core Bass program. q/k overridable for small-scale sim.

    tmode: dtype scheme for PE transposes. "bf16" streams a bf16 identity
    (1 cyc/col) against f32r-bitcast data; "f32" is the conservative
    2 cyc/col fp32 path.
    """
    import concourse.bacc as bacc
    import concourse.mybir as mybir
    import concourse.tile as tile
    from concourse.masks import make_identity

    f32 = mybir.dt.float32
    f32r = mybir.dt.float32r
    bf16 = mybir.dt.bfloat16
    AF = mybir.ActivationFunctionType

    id_dt = f32r if tmode == "f32r" else (bf16 if tmode == "bf16" else f32)
    # dtype for DMA-loaded tiles that feed PE transposes: declaring them f32r
    # at the DMA (bitcast both sides) makes the DMACopy the f32r producer
    tr_dt = f32r if tmode == "f32r" else f32

    def tr_in(dram_ap):
        return dram_ap.bitcast(f32r) if tmode == "f32r" else dram_ap

    def tp_args(data_ap, out_ap):
        if tmode in ("bf16", "f32r"):
            return data_ap.bitcast(f32r), out_ap.bitcast(f32r)
        return data_ap, out_ap

    nqc = q // 512        # 512-wide q chunks
    nkc = k // 128        # 128-wide k chunks
    nqt = q // 128        # 128-row q tiles
    ncc = C // 128        # 128-row c chunks (2)

    nc = bacc.Bacc(
        "TRN2", target_bir_lowering=False, debug=False, num_devices=N_CORES
    )

    qx_d = nc.dram_tensor("qx", [q, C], f32, kind="ExternalInput").ap()
    kvx_d = nc.dram_tensor("kvx", [k, C], f32, kind="ExternalInput").ap()
    mask_d = nc.dram_tensor("mask", [1, k], f32, kind="ExternalInput").ap()
    tri_d = nc.dram_tensor("tri", [HPC, q, k], f32, kind="ExternalInput").ap()
    wq_d = nc.dram_tensor("wq", [C, HPC * CH], f32, kind="ExternalInput").ap()
    wk_d = nc.dram_tensor("wk", [C, HPC * CH], f32, kind="ExternalInput").ap()
    wv_d = nc.dram_tensor("wv", [C, HPC * CH], f32, kind="ExternalInput").ap()
    wg_d = nc.dram_tensor("wg", [C, HPC * CH], f32, kind="ExternalInput").ap()
    bg_d = nc.dram_tensor("bg", [HPC * CH, 1], f32, kind="ExternalInput").ap()
    wo_d = nc.dram_tensor("wo", [HPC * CH, C], f32, kind="ExternalInput").ap()
    bo_d = nc.dram_tensor("bo", [1, C], f32, kind="ExternalInput").ap()
    out_d = nc.dram_tensor("out_p", [q, C], f32, kind="ExternalOutput").ap()

    inv_sqrt_ch = 1.0 / math.sqrt(CH)

    with tile.TileContext(nc) as tc:
        with (
            tc.tile_pool(name="const", bufs=1) as const,
            tc.tile_pool(name="persist", bufs=1) as persist,
        ):
            identity_f32 = const.tile([128, 128], f32)
            make_identity(nc, identity_f32)
            if id_dt == f32:
                identity = identity_f32
            else:
                identity = const.tile([128, 128], id_dt)
                nc.vector.tensor_copy(identity, identity_f32)
            wq_sb = const.tile([128, ncc * HPC * CH], f32r)
            wk_sb = const.tile([128, ncc * HPC * CH], f32r)
            wv_sb = const.tile([128, ncc * HPC * CH], f32r)
            wg_sb = const.tile([128, ncc * HPC * CH], f32r)
            wo_sb = const.tile([HPC * CH, C], f32r)
            bo_sb = const.tile([1, C], f32r)
            bg_col = const.tile([HPC * CH, 1], f32)
            bo_bc = const.tile([128, C], f32)
            # persistent activations: qT/kT hold per-head 64-row slots
            # (rows h*64 .. h*64+31 = channels, row h*64+32 = ones / mask)
            qT_sb = persist.tile([128, q], f32r)
            kT_sb = persist.tile([128, k], f32r)
            gT_sb = persist.tile([HPC * CH, q], f32)
            # per-chunk layout [v (32) | ones (1) | zeros (31)] -- padded to 64
            # so the packed AV output exactly fills a 64-row PSUM col group
            VW = 64
            v_aug = [persist.tile([128, nkc * VW], f32r, name=f"vaug{h}")
                     for h in range(HPC)]
            ogT_sb = persist.tile([HPC * CH, q], f32r)

            # ---------------- phase 0: transposes + projections ----------
            with (
                tc.tile_pool(name="ph0", bufs=1) as ph0,
                tc.tile_pool(name="ph0ps", bufs=2, space="PSUM") as ph0ps,
            ):
                qx_sb = ph0.tile([128, nqt * C], tr_dt)
                kvx_sb = ph0.tile([128, (k // 128) * C], tr_dt)
                qxT_sb = ph0.tile([128, ncc * q], f32r)
                kvxT_sb = ph0.tile([128, ncc * k], f32r)
                # x loads first (they gate everything); chunked so the first
                # transposes can start early
                for x_sb, x_d, nt in ((qx_sb, qx_d, nqt), (kvx_sb, kvx_d, k // 128)):
                    nch = 4
                    step = nt // nch
                    for ch in range(nch):
                        nc.sync.dma_start(
                            out=x_sb.rearrange("p (n c) -> p n c", c=C)[
                                :, ch * step : (ch + 1) * step, :
                            ],
                            in_=tr_in(x_d).rearrange("(n p) c -> p n c", p=128)[
                                :, ch * step : (ch + 1) * step, :
                            ],
                        )

                ones_st = ph0.tile([1, max(q, 2048)], f32)
                mask_st = ph0.tile([1, k], f32)
                ones_1x128 = ph0.tile([1, 128], f32r)
                # fp32 staging for DMA'd weights; rounded copies feed matmuls
                wq_st = ph0.tile([128, ncc * HPC * CH], f32)
                wk_st = ph0.tile([128, ncc * HPC * CH], f32)
                wv_st = ph0.tile([128, ncc * HPC * CH], f32)
                wg_st = ph0.tile([128, ncc * HPC * CH], f32)
                wo_st = ph0.tile([HPC * CH, C], f32)
                bo_st = ph0.tile([1, C], f32)
                for w_d, w_st in ((wq_d, wq_st), (wk_d, wk_st), (wv_d, wv_st), (wg_d, wg_st)):
                    nc.sync.dma_start(
                        out=w_st.rearrange("p (t m) -> p t m", t=ncc),
                        in_=w_d.rearrange("(t p) m -> p t m", p=128),
                    )
                nc.sync.dma_start(out=wo_st, in_=wo_d)
                nc.sync.dma_start(out=bo_st, in_=bo_d)
                nc.sync.dma_start(out=bg_col, in_=bg_d)
                nc.sync.dma_start(out=mask_st, in_=mask_d)
                for st, sb in ((wq_st, wq_sb), (wk_st, wk_sb), (wv_st, wv_sb),
                               (wg_st, wg_sb), (wo_st, wo_sb), (bo_st, bo_sb)):
                    nc.vector.tensor_copy(sb, st)
                nc.vector.memset(ones_st, 1.0)
                nc.vector.tensor_copy(ones_1x128, ones_st[:, :128])
                nc.vector.tensor_copy(kT_sb[32:33, :], mask_st)
                nc.vector.tensor_copy(kT_sb[96:97, :], mask_st)
                nc.vector.tensor_copy(qT_sb[32:33, :], ones_st[:, :q])
                nc.vector.tensor_copy(qT_sb[96:97, :], ones_st[:, :q])
                # b_o broadcast across partitions, used in the output phase
                pbo = ph0ps.tile([128, C], f32, tag="proj2")
                nc.tensor.matmul(pbo, ones_1x128, bo_sb)
                nc.vector.tensor_copy(bo_bc, pbo)

                # x transposes: 4 per PSUM tile, copies alternate DVE/ACT
                nxt = 0
                for x_sb, xT_sb, nt in (
                    (qx_sb, qxT_sb, nqt),
                    (kvx_sb, kvxT_sb, k // 128),
                ):
                    for n in range(nt):
                        tp = ph0ps.tile([128, 512], f32, tag="t0",
                                        name=f"tp{nxt}")
                        for cc in range(ncc):
                            xin, xout = tp_args(
                                x_sb[:, n * C + cc * 128 : n * C + cc * 128 + 128],
                                tp[:, cc * 128 : cc * 128 + 128],
                            )
                            nc.tensor.matmul(
                                xout,
                                xin,
                                identity,
                                is_transpose=True,
                                start=(cc == 0),
                                stop=(cc == ncc - 1),
                            )
                        # scatter the ncc c-chunks to their xT positions
                        for cc in range(ncc):
                            dst = xT_sb[:, cc * (nt * 128) + n * 128 :
                                        cc * (nt * 128) + n * 128 + 128]
                            src = tp[:, cc * 128 : cc * 128 + 128]
                            if nxt % 2 == 0:
                                nc.vector.tensor_copy(dst, src)
                            else:
                                nc.scalar.copy(dst, src)
                        nxt += 1

                # qT / kT projections, one head (32 rows) at a time.  Chunk 0
                # for both heads is emitted first so the main loop's first QK
                # matmuls unblock as early as possible.
                for i in range(max(nqc, k // 512)):
                    for h in range(HPC):
                        if i < nqc:
                            qn = i
                            pq = ph0ps.tile([32, 512], f32, tag="proj",
                                            name=f"pq{h}_{qn}")
                            for cc in range(ncc):
                                nc.tensor.matmul(
                                    pq,
                                    wq_sb[:, cc * 64 + h * 32 : cc * 64 + h * 32 + 32],
                                    qxT_sb[:, cc * q + qn * 512 : cc * q + qn * 512 + 512],
                                    start=(cc == 0),
                                    stop=(cc == ncc - 1),
                                )
                            nc.scalar.mul(
                                qT_sb[h * 64 : h * 64 + 32, qn * 512 : qn * 512 + 512],
                                pq,
                                inv_sqrt_ch,
                            )
                        if i < k // 512:
                            kn = i
                            pk = ph0ps.tile([32, 512], f32, tag="proj",
                                            name=f"pk{h}_{kn}")
                            for cc in range(ncc):
                                nc.tensor.matmul(
                                    pk,
                                    wk_sb[:, cc * 64 + h * 32 : cc * 64 + h * 32 + 32],
                                    kvxT_sb[:, cc * k + kn * 512 : cc * k + kn * 512 + 512],
                                    start=(cc == 0),
                                    stop=(cc == ncc - 1),
                                )
                            nc.vector.tensor_copy(
                                kT_sb[h * 64 : h * 64 + 32, kn * 512 : kn * 512 + 512],
                                pk,
                            )

                # gT (both heads at once), sigmoid with per-partition b_g
                for qn in range(nqc):
                    pg = ph0ps.tile([HPC * CH, 512], f32, tag="proj2")
                    for cc in range(ncc):
                        nc.tensor.matmul(
                            pg,
                            wg_sb[:, cc * 64 : cc * 64 + 64],
                            qxT_sb[:, cc * q + qn * 512 : cc * q + qn * 512 + 512],
                            start=(cc == 0),
                            stop=(cc == ncc - 1),
                        )
                    nc.scalar.activation(
                        gT_sb[:, qn * 512 : qn * 512 + 512],
                        pg,
                        AF.Sigmoid,
                        bias=bg_col,
                    )

                # v natural [k, 64] -> per-head augmented [128, nkc*(CH+1)]
                for kn in range(nkc):
                    pv = ph0ps.tile([128, HPC * CH], f32, tag="projv")
                    for cc in range(ncc):
                        nc.tensor.matmul(
                            pv,
                            kvxT_sb[:, cc * k + kn * 128 : cc * k + kn * 128 + 128],
                            wv_sb[:, cc * 64 : cc * 64 + 64],
                            start=(cc == 0),
                            stop=(cc == ncc - 1),
                        )
                    for h in range(HPC):
                        nc.vector.tensor_copy(
                            v_aug[h][:, kn * VW : kn * VW + CH],
                            pv[:, h * CH : h * CH + CH],
                        )
                # fill [ones | zeros] tail columns of v_aug via one fp32
                # staging tile (memset cannot produce f32r directly)
                vfill = ph0.tile([128, nkc * (VW - CH)], f32)
                vfill3 = vfill.rearrange("p (n c) -> p n c", c=VW - CH)
                nc.vector.memset(vfill, 0.0)
                nc.vector.memset(vfill3[:, :, 0], 1.0)
                for h in range(HPC):
                    va3 = v_aug[h].rearrange("p (n c) -> p n c", c=VW)
                    nc.vector.tensor_copy(va3[:, :, CH:VW], vfill3)

            # ---------------- main loop ---------------------------------
            # Heads are interleaved per (qc, kc2) so the two heads' QK and AV
            # matmuls sit adjacent in the PE stream with different
            # tile_positions (row groups 0/64 for QK, col groups 0/64 for AV)
            # and execute concurrently in the array's quadrants.
            khalf = nkc // 2
            with (
                tc.tile_pool(name="bias", bufs=18) as biasp,
                tc.tile_pool(name="attn", bufs=1) as attnp,
                tc.tile_pool(name="small", bufs=2) as smallp,
                tc.tile_pool(name="mainps", bufs=1, space="PSUM") as mps,
            ):
                for qc in range(nqc):
                    bts = {}
                    for kh in range(2):
                        for h in range(HPC):
                            for j in range(4):
                                bt = biasp.tile(
                                    [128, k // 2], tr_dt, tag="bias",
                                    name=f"bt{qc}_{kh}_{h}_{j}",
                                )
                                nc.sync.dma_start(
                                    out=bt,
                                    in_=tr_in(tri_d)[
                                        h,
                                        qc * 512 + j * 128 : qc * 512 + j * 128 + 128,
                                        kh * (k // 2) : (kh + 1) * (k // 2),
                                    ],
                                )
                                bts[(kh, h, j)] = bt
                    attnT = [
                        attnp.tile([128, nkc * 512], f32r, tag=f"attnT{h}",
                                   name=f"attnT{h}_{qc}")
                        for h in range(HPC)
                    ]
                    for kc2 in range(khalf):
                        Ss = []
                        for h in range(HPC):
                            S2 = mps.tile([128, 1024], f32, tag="S", bufs=3,
                                          name=f"S{h}_{qc}_{kc2}")
                            Ss.append(S2)
                            for t in range(2):
                                kc = kc2 * 2 + t
                                kh, kcol = divmod(kc, khalf)
                                for j in range(4):
                                    bin_, bout = tp_args(
                                        bts[(kh, h, j)][:, kcol * 128 : kcol * 128 + 128],
                                        S2[:, t * 512 + j * 128 : t * 512 + j * 128 + 128],
                                    )
                                    nc.tensor.matmul(
                                        bout,
                                        bin_,
                                        identity,
                                        is_transpose=True,
                                        start=(j == 0),
                                        stop=False,
                                    )
                        for t in range(2):
                            kc = kc2 * 2 + t
                            for h in range(HPC):
                                hb = h * 64
                                nc.tensor.matmul(
                                    Ss[h][:, t * 512 : t * 512 + 512],
                                    kT_sb[hb : hb + 33, kc * 128 : kc * 128 + 128],
                                    qT_sb[hb : hb + 33, qc * 512 : qc * 512 + 512],
                                    start=False,
                                    stop=True,
                                )
                        for h in range(HPC):
                            nc.scalar.activation(
                                attnT[h][:, kc2 * 1024 : kc2 * 1024 + 1024],
                                Ss[h],
                                AF.Exp,
                            )
                    # AV with fused denominator (ones column of v_aug);
                    # per-head PSUM tiles (walrus requires matmul dst at
                    # partition base 0)
                    o_aug = [
                        mps.tile([64, 512], f32, tag=f"av{h}", bufs=1,
                                 name=f"oaug{qc}_{h}")
                        for h in range(HPC)
                    ]
                    for kc in range(nkc):
                        for h in range(HPC):
                            nc.tensor.matmul(
                                o_aug[h][:, :],
                                v_aug[h][:, kc * VW : (kc + 1) * VW],
                                attnT[h][:, kc * 512 : kc * 512 + 512],
                                start=(kc == 0),
                                stop=(kc == nkc - 1),
                            )
                    # SBUF-SBUF tensor_tensor inputs must share a start
                    # partition, so the broadcast / gating tiles are sliced at
                    # each head's base (h*32) to line up with gT_sb / ogT_sb
                    rbc_full = smallp.tile([HPC * CH, 512], f32, tag="rbc",
                                           bufs=2, name=f"rbc{qc}")
                    gtmp_full = smallp.tile([HPC * CH, 512], f32, tag="gtmp",
                                            bufs=2, name=f"gtmp{qc}")
                    for h in range(HPC):
                        recip_f = smallp.tile([1, 512], f32, tag=f"recipf{h}",
                                              bufs=1, name=f"recf{qc}_{h}")
                        nc.vector.reciprocal(recip_f, o_aug[h][CH : CH + 1, :])
                        # broadcast 1/sum across 32 partitions on the (idle)
                        # GPSIMD engine instead of spending PSUM + PE on it.
                        # partition_broadcast only writes correctly at
                        # partition base 0, so h1 goes through a bounce tile.
                        r_bc = rbc_full[h * CH : h * CH + CH, :]
                        if h == 0:
                            nc.gpsimd.partition_broadcast(r_bc, recip_f)
                        else:
                            bc_tmp = smallp.tile([CH, 512], f32, tag="bctmp",
                                                 bufs=1, name=f"bct{qc}_{h}")
                            nc.gpsimd.partition_broadcast(bc_tmp, recip_f)
                            nc.vector.tensor_copy(r_bc, bc_tmp)
                        gtmp = gtmp_full[h * CH : h * CH + CH, :]
                        nc.vector.tensor_mul(
                            gtmp,
                            gT_sb[h * CH : h * CH + CH, qc * 512 : qc * 512 + 512],
                            r_bc,
                        )
                        nc.vector.tensor_mul(
                            ogT_sb[h * CH : h * CH + CH, qc * 512 : qc * 512 + 512],
                            gtmp,
                            o_aug[h][0:CH, :],
                        )
                    # output projection for this q block, pipelined with the
                    # next block's scores
                    ob = smallp.tile([128, 4 * C], f32, tag="ob",
                                     name=f"ob{qc}")
                    for s in range(4):
                        qs = qc * 4 + s
                        op = mps.tile([128, C], f32, tag="av1", bufs=1,
                                      name=f"op{qc}_{s}")
                        nc.tensor.matmul(
                            op,
                            ogT_sb[:, qs * 128 : qs * 128 + 128],
                            wo_sb,
                        )
                        nc.vector.tensor_add(ob[:, s * C : s * C + C], op, bo_bc)
                    nc.sync.dma_start(
                        out=out_d[qc * 512 : qc * 512 + 512, :].rearrange(
                            "(n p) c -> p n c", p=128
                        ),
                        in_=ob.rearrange("p (n c) -> p n c", c=C),
                    )
    nc.compile()
    return nc


def _shard_inputs(q_x, kv_x, mask_bias, triangle_bias, w_q, w_k, w_v, w_g,
                  b_g, w_o, b_o):
    """Build the 8 per-core input maps."""
    in_maps = []
    for core in range(N_CORES):
        b = core // GROUPS
        g = core % GROUPS
        h0 = g * HPC
        cs = slice(h0 * CH, (h0 + HPC) * CH)
        bo = b_o if g == 0 else np.zeros_like(b_o)
        in_maps.append({
            "qx": np.ascontiguousarray(q_x[b]),
            "kvx": np.ascontiguousarray(kv_x[b]),
            "mask": np.ascontiguousarray(mask_bias[b, 0, 0]).reshape(1, K),
            "tri": np.ascontiguousarray(triangle_bias[b, h0 : h0 + HPC]),
            "wq": np.ascontiguousarray(w_q[:, cs]),
            "wk": np.ascontiguousarray(w_k[:, cs]),
            "wv": np.ascontiguousarray(w_v[:, cs]),
            "wg": np.ascontiguousarray(w_g[:, cs]),
            "bg": np.ascontiguousarray(b_g[cs]).reshape(HPC * CH, 1),
            "wo": np.ascontiguousarray(w_o[cs, :]),
            "bo": np.ascontiguousarray(bo).reshape(1, C),
        })
    return in_maps


def kernel(**inputs):
    from concourse import bass_utils

    inputs = {k_: np.asarray(v, dtype=np.float32) for k_, v in inputs.items()}
    if "nc" not in _cache:
        _cache["nc"] = _build_nc()
    nc = _cache["nc"]

    in_maps = _shard_inputs(**inputs)
    res = bass_utils.run_bass_kernel_spmd(nc, in_maps, core_ids=list(range(N_CORES)))

    out = np.zeros((B, Q, C), np.float32)
    for core in range(N_CORES):
        out[core // GROUPS] += res.results[core]["out_p"]
    return out



# revision 10
# speedup vs baseline: 1.4027x; 1.4027x over previous
"""Trainium2 Bass kernel for a gated cross-attention block with a dense
per-(b,h) attention bias (B=2, Q=K=2048, C=256, H=8, CH=32).

Sharding: the 16 (batch, 2-head group) units are data-parallel across 8
cores: core i handles batch b = i//4 and heads h0 = 2*(i%4), h0+1.  Linear
weights are column-sliced per head group; the output projection is
tensor-parallel over H*CH, so each core emits a partial [Q, C] output and the
host sums the 4 partials per batch (b_o is added once on the host).

The dense bias never touches the PE or a vector-add path.  The host
precomputes expb = exp(triangle_bias + mask_bias) transposed to [k, q] fp16
(half the DMA bytes of f32, already in the layout the transposed-scores
dataflow wants) and the device uses
    softmax(s + b) ∝ exp(s) * expb
so applying the bias is one fp16 DVE multiply in the 2x perf mode.  All
matmul operands are fp16 (1 PE cycle/column), scores accumulate in PSUM f32,
and the softmax denominator falls out of the AV matmul via an appended ones
column in v.

Per-core dataflow:
  phase 0: host-pretransposed qxT/kvxT fp16 are DMA'd; projections produce
    qT/kT [64, Q] fp16 (head h in rows h*32..), v_aug [128, h|kc|33] fp16
    with the ones column fused, and sigmoid gates gT.
  main loop, per 512-wide q block (qc): scoresT tiles [128 k, 1024=2x512 q]
    are built in PSUM ([128,1024] S tiles, ring of 3), exp'd on ACT to fp16,
    multiplied by the prefetched expb tile on DVE (2x mode), and consumed by
    AV matmuls that lag ~2 chunks behind.  Gating (reciprocal + partition
    broadcast + gate multiply) and the output projection for block qc-1 are
    interleaved into block qc's instruction streams so the ACT engine (the
    pacer at ~16.6us/block) never stalls.
"""

import math

import numpy as np

B, Q, K, C, H, CH = 2, 2048, 2048, 256, 8, 32
N_CORES = 8
HPC = 2            # heads per core
GROUPS = H // HPC  # head groups per batch = 4

_cache = {}


def _build_nc(q=Q, k=K):
    import concourse.bacc as bacc
    import concourse.mybir as mybir
    import concourse.tile as tile

    f32 = mybir.dt.float32
    f16 = mybir.dt.float16
    AF = mybir.ActivationFunctionType

    nqc = q // 512        # 512-wide q chunks (4)
    nkt = k // 128        # 128-row k tiles (16)
    nkc2 = nkt // 2       # 1024-wide score chunks per q block (8)
    ncc = C // 128        # 128-row c chunks (2)
    HW = HPC * CH         # 64

    nc = bacc.Bacc(
        "TRN2", target_bir_lowering=False, debug=False, num_devices=N_CORES
    )

    qxT_d = nc.dram_tensor("qxT", [C, q], f16, kind="ExternalInput").ap()
    kvxT_d = nc.dram_tensor("kvxT", [C, k], f16, kind="ExternalInput").ap()
    eb_d = [
        nc.dram_tensor(f"eb{h}", [k, q], f16, kind="ExternalInput").ap()
        for h in range(HPC)
    ]
    wq_d = nc.dram_tensor("wq", [C, HW], f16, kind="ExternalInput").ap()
    wk_d = nc.dram_tensor("wk", [C, HW], f16, kind="ExternalInput").ap()
    wv_d = nc.dram_tensor("wv", [C, HW], f16, kind="ExternalInput").ap()
    wg_d = nc.dram_tensor("wg", [C, HW], f16, kind="ExternalInput").ap()
    bg_d = nc.dram_tensor("bg", [CH, HPC], f32, kind="ExternalInput").ap()
    wo_d = nc.dram_tensor("wo", [CH, HPC * C], f16, kind="ExternalInput").ap()
    out_d = nc.dram_tensor("out_p", [q, C], f32, kind="ExternalOutput").ap()

    with tile.TileContext(nc) as tc:
        with (
            tc.tile_pool(name="const", bufs=1) as const,
            tc.tile_pool(name="persist", bufs=1) as persist,
            tc.tile_pool(name="ebp", bufs=2) as ebp,
            tc.tile_pool(name="attp", bufs=1) as attp,
            tc.tile_pool(name="app", bufs=3) as app,
            tc.tile_pool(name="small", bufs=1) as smallp,
            tc.tile_pool(name="obp", bufs=1) as obp,
        ):
            # ---------------- persistent SBUF tiles -----------------------
            wq_sb = const.tile([128, ncc * HW], f16)
            wk_sb = const.tile([128, ncc * HW], f16)
            wv_sb = const.tile([128, ncc * HW], f16)
            wg_sb = const.tile([128, ncc * HW], f16)
            wo_sb = const.tile([CH, HPC * C], f16)
            bg_sb = const.tile([CH, HPC], f32)

            qT = persist.tile([HW, q], f16)     # rows h*32.. : head h
            kT = persist.tile([HW, k], f16)
            gT = persist.tile([CH, HPC * q], f16)   # cols h*q.. : head h
            ogT = persist.tile([CH, HPC * q], f16)
            VW = CH + 1
            vA = persist.tile([128, HPC * nkt * VW], f16)  # [v(32)|ones]
            vA4 = vA.rearrange("p (h n c) -> p h n c", h=HPC, c=VW)
            ones_st = persist.tile([128, CH], f32)

            # expb prefetch: [128, nkt*512] fp16 per (head, qc), ring of 2,
            # two DMA pieces each for earlier availability
            def emit_eb_dma(qc):
                tiles = []
                for h in range(HPC):
                    t = ebp.tile([128, nkt * 512], f16, tag=f"eb{h}",
                                 name=f"eb{h}_{qc}")
                    t3 = t.rearrange("p (n c) -> p n c", c=512)
                    src = eb_d[h].rearrange("(n p) m -> p n m", p=128)
                    half = nkt // 2
                    for piece in range(2):
                        sl = slice(piece * half, (piece + 1) * half)
                        nc.sync.dma_start(
                            out=t3[:, sl, :],
                            in_=src[:, sl, qc * 512:qc * 512 + 512],
                        )
                    tiles.append(t)
                return tiles

            eb_tiles = {0: emit_eb_dma(0)}

            # ---------------- phase 0: projections ------------------------
            with (
                tc.tile_pool(name="ph0sb", bufs=1) as ph0sb,
                tc.tile_pool(name="ph0ps", bufs=2, space="PSUM") as ph0ps,
            ):
                qxT_sb = ph0sb.tile([128, ncc * q], f16)
                kvxT_sb = ph0sb.tile([128, ncc * k], f16)
                for x_sb, x_d, n in ((qxT_sb, qxT_d, q), (kvxT_sb, kvxT_d, k)):
                    nc.sync.dma_start(
                        out=x_sb.rearrange("p (t n) -> p t n", t=ncc),
                        in_=x_d.rearrange("(t p) n -> p t n", p=128),
                    )
                for w_sb, w_d in ((wq_sb, wq_d), (wk_sb, wk_d),
                                  (wv_sb, wv_d), (wg_sb, wg_d)):
                    nc.sync.dma_start(
                        out=w_sb.rearrange("p (t m) -> p t m", t=ncc),
                        in_=w_d.rearrange("(t p) m -> p t m", p=128),
                    )
                nc.sync.dma_start(out=wo_sb, in_=wo_d)
                nc.sync.dma_start(out=bg_sb, in_=bg_d)

                # qT / kT: both heads at once, [64, 512] PSUM chunks; copies
                # alternate DVE/ACT
                nxt = 0
                for i in range(max(nqc, k // 512)):
                    for src_sb, w_sb_, dst, nn in (
                        (qxT_sb, wq_sb, qT, q), (kvxT_sb, wk_sb, kT, k)
                    ):
                        if i >= nn // 512:
                            continue
                        p = ph0ps.tile([HW, 512], f32, tag="proj", bufs=3,
                                       name=f"p{nxt}")
                        for cc in range(ncc):
                            nc.tensor.matmul(
                                p,
                                w_sb_[:, cc * HW:cc * HW + HW],
                                src_sb[:, cc * nn + i * 512:
                                       cc * nn + i * 512 + 512],
                                start=(cc == 0),
                                stop=(cc == ncc - 1),
                            )
                        if nxt % 2 == 0:
                            nc.vector.tensor_copy(
                                dst[:, i * 512:i * 512 + 512], p)
                        else:
                            nc.scalar.copy(dst[:, i * 512:i * 512 + 512], p)
                        nxt += 1

                # v (natural orientation), both heads per 128-row k tile
                for kn in range(nkt):
                    pv = ph0ps.tile([128, HW], f32, tag="projv", bufs=2,
                                    name=f"pv{kn}")
                    for cc in range(ncc):
                        nc.tensor.matmul(
                            pv,
                            kvxT_sb[:, cc * k + kn * 128:
                                    cc * k + kn * 128 + 128],
                            wv_sb[:, cc * HW:cc * HW + HW],
                            start=(cc == 0),
                            stop=(cc == ncc - 1),
                        )
                    nc.vector.tensor_copy(
                        vA4[:, :, kn, 0:CH],
                        pv.rearrange("p (h c) -> p h c", h=HPC),
                    )
                # ones columns of v_aug (softmax denominator trick)
                nc.vector.memset(ones_st, 1.0)
                for h in range(HPC):
                    nc.vector.tensor_copy(
                        vA4[:, h, :, CH:VW],
                        ones_st[:, 0:nkt].rearrange("p (n c) -> p n c", c=1),
                    )

                # gT: sigmoid(x@wg + bg), per head (partition base 0)
                for qn in range(nqc):
                    for h in range(HPC):
                        pg = ph0ps.tile([CH, 512], f32, tag="projg", bufs=2,
                                        name=f"pg{qn}_{h}")
                        for cc in range(ncc):
                            nc.tensor.matmul(
                                pg,
                                wg_sb[:, cc * HW + h * CH:
                                      cc * HW + h * CH + CH],
                                qxT_sb[:, cc * q + qn * 512:
                                       cc * q + qn * 512 + 512],
                                start=(cc == 0),
                                stop=(cc == ncc - 1),
                            )
                        nc.scalar.activation(
                            gT[:, h * q + qn * 512:h * q + qn * 512 + 512],
                            pg,
                            AF.Sigmoid,
                            bias=bg_sb[:, h:h + 1],
                        )

            # ---------------- main loop -----------------------------------
            with tc.tile_pool(name="mainps", bufs=1, space="PSUM") as mps:
                o_aug = {}       # (qc, h) -> [33, 512] PSUM accumulator
                attn_map = {}    # qc -> per-head (attnA, attnB) tile pairs
                pend = []        # pending AV units (qc, h, kc)
                KSPLIT = nkt - 4  # k tiles >= KSPLIT cross into the next
                #                   iteration -> double-buffered tail tile

                def attn_ap(uqc, h, kc):
                    a, bt = attn_map[uqc][h]
                    if kc < KSPLIT:
                        return a[:, kc * 512:kc * 512 + 512]
                    return bt[:, (kc - KSPLIT) * 512:(kc - KSPLIT) * 512 + 512]

                def emit_av(uqc, h, kc):
                    if (uqc, h) not in o_aug:
                        o_aug[(uqc, h)] = mps.tile(
                            [VW, 512], f32, tag=f"av{h}", bufs=1,
                            name=f"oaug{uqc}_{h}")
                    nc.tensor.matmul(
                        o_aug[(uqc, h)],
                        vA4[:, h, kc, :],
                        attn_ap(uqc, h, kc),
                        start=(kc == 0),
                        stop=(kc == nkt - 1),
                    )

                def emit_gating(gqc):
                    # reciprocal of the denominator row; o_aug copied out to
                    # SBUF fp16 (frees PSUM early); broadcast 1/den over 32
                    # partitions on Pool; gate+normalize
                    o_sb = smallp.tile([VW, HPC * 512], f16, tag="osb",
                                       name=f"osb{gqc}")
                    recip = smallp.tile([1, HPC * 512], f32, tag="recip",
                                        name=f"recip{gqc}")
                    for h in range(HPC):
                        nc.vector.reciprocal(
                            recip[:, h * 512:h * 512 + 512],
                            o_aug[(gqc, h)][CH:CH + 1, :],
                        )
                        # GPSIMD cannot read PSUM -> DVE does this copy
                        nc.vector.tensor_copy(
                            o_sb[:, h * 512:h * 512 + 512], o_aug[(gqc, h)]
                        )
                        del o_aug[(gqc, h)]
                    r_bc = smallp.tile([CH, HPC * 512], f32, tag="rbc",
                                       name=f"rbc{gqc}")
                    nc.gpsimd.partition_broadcast(r_bc, recip)
                    gtmp = smallp.tile([CH, HPC * 512], f32, tag="gtmp",
                                       name=f"gtmp{gqc}")
                    for h in range(HPC):
                        nc.gpsimd.tensor_mul(
                            gtmp[:, h * 512:h * 512 + 512],
                            gT[:, h * q + gqc * 512:h * q + gqc * 512 + 512],
                            r_bc[:, h * 512:h * 512 + 512],
                        )
                    for h in range(HPC):
                        # all-SBUF multiply -> Pool (keeps DVE for the
                        # PSUM-reading work it alone can do)
                        nc.gpsimd.tensor_mul(
                            ogT[:, h * q + gqc * 512:h * q + gqc * 512 + 512],
                            gtmp[:, h * 512:h * 512 + 512],
                            o_sb[0:CH, h * 512:h * 512 + 512],
                        )

                def emit_proj(pqc):
                    # output projection for block pqc; rides the S ring so
                    # PSUM stays within 8 banks
                    op = mps.tile([128, 1024], f32, tag="S", bufs=3,
                                  name=f"op{pqc}")
                    for s in range(4):
                        qs = pqc * 4 + s
                        for h in range(HPC):
                            nc.tensor.matmul(
                                op[:, s * 256:s * 256 + 256],
                                ogT[:, h * q + qs * 128:h * q + qs * 128 + 128],
                                wo_sb[:, h * C:h * C + C],
                                start=(h == 0),
                                stop=(h == HPC - 1),
                            )
                    ob = obp.tile([128, 1024], f32, tag="ob", name=f"ob{pqc}")
                    nc.vector.tensor_copy(ob, op)
                    nc.sync.dma_start(
                        out=out_d[pqc * 512:pqc * 512 + 512, :].rearrange(
                            "(n p) c -> p n c", p=128
                        ),
                        in_=ob.rearrange("p (n c) -> p n c", c=C),
                    )

                def drain_av(cur_qc, kc2, limit=6):
                    ready = [u for u in pend if u[0] < cur_qc] + [
                        u for u in pend
                        if u[0] == cur_qc and u[2] < kc2 * 2 - 2
                    ]
                    for u in ready[:limit]:
                        pend.remove(u)
                        emit_av(*u)

                for qc in range(nqc):
                    if qc + 1 < nqc:
                        eb_tiles[qc + 1] = emit_eb_dma(qc + 1)
                    ebt = eb_tiles.pop(qc)
                    attn_map[qc] = [
                        (attp.tile([128, KSPLIT * 512], f16, tag=f"attnA{h}",
                                   bufs=1, name=f"attnA{h}_{qc}"),
                         attp.tile([128, (nkt - KSPLIT) * 512], f16,
                                   tag=f"attnB{h}", bufs=2,
                                   name=f"attnB{h}_{qc}"))
                        for h in range(HPC)
                    ]

                    for kc2 in range(nkc2):
                        for h in range(HPC):
                            S = mps.tile([128, 1024], f32, tag="S", bufs=3,
                                         name=f"S{qc}_{kc2}_{h}")
                            for t in range(2):
                                kc = kc2 * 2 + t
                                nc.tensor.matmul(
                                    S[:, t * 512:t * 512 + 512],
                                    kT[h * CH:h * CH + CH,
                                       kc * 128:kc * 128 + 128],
                                    qT[h * CH:h * CH + CH,
                                       qc * 512:qc * 512 + 512],
                                )
                            ap_t = app.tile([128, 1024], f16, tag="ap",
                                            name=f"ap{qc}_{kc2}_{h}")
                            nc.scalar.activation(ap_t, S, AF.Exp)
                            if kc2 * 2 < KSPLIT:
                                mdst = attn_map[qc][h][0][
                                    :, kc2 * 1024:kc2 * 1024 + 1024]
                            else:
                                off = kc2 * 2 - KSPLIT
                                mdst = attn_map[qc][h][1][
                                    :, off * 512:off * 512 + 1024]
                            nc.vector.tensor_mul(
                                mdst,
                                ap_t,
                                ebt[h][:, kc2 * 1024:kc2 * 1024 + 1024],
                            )
                            for t in range(2):
                                pend.append((qc, h, kc2 * 2 + t))
                        # gating for the previous block goes BEFORE this
                        # slot's AV drain so the o_aug ring (bufs=1) sees its
                        # reads emitted before the next block's first write
                        if qc > 0 and kc2 == 2:
                            emit_gating(qc - 1)
                        drain_av(qc, kc2)
                        if qc > 0 and kc2 == 5:
                            emit_proj(qc - 1)
                            del attn_map[qc - 1]

                # drain: AV tail for the last block, then gating + proj
                last = nqc - 1
                for u in list(pend):
                    pend.remove(u)
                    emit_av(*u)
                emit_gating(last)
                emit_proj(last)

    nc.compile()
    return nc


def _shard_inputs(q_x, kv_x, mask_bias, triangle_bias, w_q, w_k, w_v, w_g,
                  b_g, w_o, b_o):
    """Build the 8 per-core input maps (host-side layout + precompute)."""
    f16 = np.float16
    inv = 1.0 / math.sqrt(CH)
    in_maps = []
    for core in range(N_CORES):
        b = core // GROUPS
        g = core % GROUPS
        h0 = g * HPC
        cs = slice(h0 * CH, (h0 + HPC) * CH)
        m = {
            "qxT": np.ascontiguousarray(q_x[b].T).astype(f16),
            "kvxT": np.ascontiguousarray(kv_x[b].T).astype(f16),
            "wq": (w_q[:, cs] * inv).astype(f16),
            "wk": w_k[:, cs].astype(f16),
            "wv": w_v[:, cs].astype(f16),
            "wg": w_g[:, cs].astype(f16),
            "bg": np.ascontiguousarray(
                b_g[cs].reshape(HPC, CH).T).astype(np.float32),
            "wo": np.ascontiguousarray(
                w_o[cs, :].reshape(HPC, CH, C).transpose(1, 0, 2)
            ).reshape(CH, HPC * C).astype(f16),
        }
        mk = mask_bias[b, 0, 0]  # [K]
        for h in range(HPC):
            eb = np.exp(triangle_bias[b, h0 + h] + mk[None, :])
            m[f"eb{h}"] = np.ascontiguousarray(eb.T).astype(f16)
        in_maps.append(m)
    return in_maps


def kernel(**inputs):
    from concourse import bass_utils

    inputs = {k_: np.asarray(v, dtype=np.float32) for k_, v in inputs.items()}
    if "nc" not in _cache:
        _cache["nc"] = _build_nc()
    nc = _cache["nc"]

    in_maps = _shard_inputs(**inputs)
    res = bass_utils.run_bass_kernel_spmd(nc, in_maps,
                                          core_ids=list(range(N_CORES)))

    out = np.zeros((B, Q, C), np.float32)
    for core in range(N_CORES):
        out[core // GROUPS] += res.results[core]["out_p"]
    out += inputs["b_o"][None, None, :]
    return out


# revision 17
# speedup vs baseline: 1.6035x; 1.1431x over previous
"""Trainium2 Bass kernel for a gated cross-attention block with a dense
per-(b,h) attention bias (B=2, Q=K=2048, C=256, H=8, CH=32).

Sharding: the 16 (batch, 2-head group) units are data-parallel across 8
cores: core i handles batch b = i//4 and heads h0 = 2*(i%4), h0+1.  Linear
weights are column-sliced per head group; the output projection is
tensor-parallel over H*CH, so each core emits a partial [Q, C] output and the
host sums the 4 partials per batch (b_o is added once on the host).

The dense bias never touches the PE or a vector-add path.  The host
precomputes expb = exp(triangle_bias + mask_bias) transposed to [k, q] fp16
(half the DMA bytes of f32, already in the layout the transposed-scores
dataflow wants) and the device uses
    softmax(s + b) ∝ exp(s) * expb
so applying the bias is one fp16 DVE multiply in the 2x perf mode.  All
matmul operands are fp16 (1 PE cycle/column), scores accumulate in PSUM f32,
and the softmax denominator falls out of the AV matmul via an appended ones
column in v.

Per-core dataflow:
  phase 0: host-pretransposed qxT/kvxT fp16 are DMA'd; projections produce
    qT/kT [64, Q] fp16 (head h in rows h*32..), v_aug [128, h|kc|33] fp16
    with the ones column fused, and sigmoid gates gT.
  main loop, per 512-wide q block (qc): scoresT tiles [128 k, 1024=2x512 q]
    are built in PSUM ([128,1024] S tiles, ring of 3), exp'd on ACT to fp16,
    multiplied by the prefetched expb tile on DVE (2x mode), and consumed by
    AV matmuls that lag ~2 chunks behind.  Gating (reciprocal + partition
    broadcast + gate multiply) and the output projection for block qc-1 are
    interleaved into block qc's instruction streams so the ACT engine (the
    pacer at ~16.6us/block) never stalls.
"""

import math

import numpy as np

B, Q, K, C, H, CH = 2, 2048, 2048, 256, 8, 32
N_CORES = 8
HPC = 2            # heads per core
GROUPS = H // HPC  # head groups per batch = 4

_cache = {}


def _build_nc(q=Q, k=K):
    import concourse.bacc as bacc
    import concourse.mybir as mybir
    import concourse.tile as tile

    f32 = mybir.dt.float32
    f16 = mybir.dt.float16
    AF = mybir.ActivationFunctionType

    nqc = q // 512        # 512-wide q chunks (4)
    nkt = k // 128        # 128-row k tiles (16)
    nkc2 = nkt // 2       # 1024-wide score chunks per q block (8)
    ncc = C // 128        # 128-row c chunks (2)
    HW = HPC * CH         # 64

    nc = bacc.Bacc(
        "TRN2", target_bir_lowering=False, debug=False, num_devices=N_CORES
    )

    qxT_d = nc.dram_tensor("qxT", [C, q], f16, kind="ExternalInput").ap()
    kvxT_d = nc.dram_tensor("kvxT", [C, k], f16, kind="ExternalInput").ap()
    eb_d = [
        nc.dram_tensor(f"eb{h}", [k, q], f16, kind="ExternalInput").ap()
        for h in range(HPC)
    ]
    wq_d = nc.dram_tensor("wq", [C, HW], f16, kind="ExternalInput").ap()
    wk_d = nc.dram_tensor("wk", [C, HW], f16, kind="ExternalInput").ap()
    wv_d = nc.dram_tensor("wv", [C, HW], f16, kind="ExternalInput").ap()
    wg_d = nc.dram_tensor("wg", [C, HW], f16, kind="ExternalInput").ap()
    bg_d = nc.dram_tensor("bg", [CH, HPC], f32, kind="ExternalInput").ap()
    wo_d = nc.dram_tensor("wo", [CH, HPC * C], f16, kind="ExternalInput").ap()
    out_d = nc.dram_tensor("out_p", [q, C], f32, kind="ExternalOutput").ap()

    with tile.TileContext(nc) as tc:
        with (
            tc.tile_pool(name="const", bufs=1) as const,
            tc.tile_pool(name="persist", bufs=1) as persist,
            tc.tile_pool(name="ebp", bufs=2) as ebp,
            tc.tile_pool(name="attp", bufs=1) as attp,
            tc.tile_pool(name="app", bufs=4) as app,
            tc.tile_pool(name="small", bufs=1) as smallp,
            tc.tile_pool(name="obp", bufs=1) as obp,
        ):
            # ---------------- persistent SBUF tiles -----------------------
            wq_sb = const.tile([128, ncc * HW], f16)
            wk_sb = const.tile([128, ncc * HW], f16)
            wv_sb = const.tile([128, ncc * HW], f16)
            wg_sb = const.tile([128, ncc * HW], f16)
            wo_sb = const.tile([CH, HPC * C], f16)
            bg_sb = const.tile([CH, HPC], f32)

            qT = persist.tile([HW, q], f16)     # rows h*32.. : head h
            kT = persist.tile([HW, k], f16)
            gT = persist.tile([CH, HPC * q], f16)   # cols h*q.. : head h
            ogT = persist.tile([CH, HPC * q], f16)
            VW = CH + 1
            vA = persist.tile([128, HPC * nkt * VW], f16)  # [v(32)|ones]
            vA4 = vA.rearrange("p (h n c) -> p h n c", h=HPC, c=VW)
            ones_st = persist.tile([128, CH], f32)

            # expb prefetch: [128, nkt*512] fp16 per (head, qc), ring of 2,
            # two DMA pieces each for earlier availability
            def emit_eb_dma(qc, npiece=2):
                # pieces interleaved across heads so the consumer (DVE mult,
                # h-inner order) sees both heads' early chunks early
                tiles = [
                    ebp.tile([128, nkt * 512], f16, tag=f"eb{h}",
                             name=f"eb{h}_{qc}")
                    for h in range(HPC)
                ]
                step = nkt // npiece
                for piece in range(npiece):
                    sl = slice(piece * step, (piece + 1) * step)
                    for h in range(HPC):
                        nc.sync.dma_start(
                            out=tiles[h].rearrange(
                                "p (n c) -> p n c", c=512)[:, sl, :],
                            in_=eb_d[h].rearrange(
                                "(n p) m -> p n m", p=128
                            )[:, sl, qc * 512:qc * 512 + 512],
                        )
                return tiles

            # ---------------- phase 0: projections ------------------------
            with (
                tc.tile_pool(name="ph0sb", bufs=1) as ph0sb,
                tc.tile_pool(name="ph0ps", bufs=2, space="PSUM") as ph0ps,
            ):
                qxT_sb = ph0sb.tile([128, ncc * q], f16)
                kvxT_sb = ph0sb.tile([128, ncc * k], f16)
                # DMA order matters: everything the first QK needs (weights,
                # then x) goes before the big expb streams
                for w_sb, w_d in ((wq_sb, wq_d), (wk_sb, wk_d)):
                    nc.sync.dma_start(
                        out=w_sb.rearrange("p (t m) -> p t m", t=ncc),
                        in_=w_d.rearrange("(t p) m -> p t m", p=128),
                    )
                for x_sb, x_d, n in ((qxT_sb, qxT_d, q), (kvxT_sb, kvxT_d, k)):
                    nc.sync.dma_start(
                        out=x_sb.rearrange("p (t n) -> p t n", t=ncc),
                        in_=x_d.rearrange("(t p) n -> p t n", p=128),
                    )
                for w_sb, w_d in ((wv_sb, wv_d), (wg_sb, wg_d)):
                    nc.sync.dma_start(
                        out=w_sb.rearrange("p (t m) -> p t m", t=ncc),
                        in_=w_d.rearrange("(t p) m -> p t m", p=128),
                    )
                nc.sync.dma_start(out=wo_sb, in_=wo_d)
                nc.sync.dma_start(out=bg_sb, in_=bg_d)
                eb_tiles = {0: emit_eb_dma(0, npiece=4)}

                # qT / kT: both heads at once, [64, 512] PSUM chunks; copies
                # alternate DVE/ACT
                nxt = 0
                for i in range(max(nqc, k // 512)):
                    for src_sb, w_sb_, dst, nn in (
                        (qxT_sb, wq_sb, qT, q), (kvxT_sb, wk_sb, kT, k)
                    ):
                        if i >= nn // 512:
                            continue
                        p = ph0ps.tile([HW, 512], f32, tag="proj", bufs=3,
                                       name=f"p{nxt}")
                        for cc in range(ncc):
                            nc.tensor.matmul(
                                p,
                                w_sb_[:, cc * HW:cc * HW + HW],
                                src_sb[:, cc * nn + i * 512:
                                       cc * nn + i * 512 + 512],
                                start=(cc == 0),
                                stop=(cc == ncc - 1),
                            )
                        # DVE only: ACT is the main-loop pacer and its
                        # phase-0 slack is wanted for the sigmoids
                        nc.vector.tensor_copy(
                            dst[:, i * 512:i * 512 + 512], p)
                        nxt += 1

                # v (natural orientation), both heads per 128-row k tile
                for kn in range(nkt):
                    pv = ph0ps.tile([128, HW], f32, tag="projv", bufs=2,
                                    name=f"pv{kn}")
                    for cc in range(ncc):
                        nc.tensor.matmul(
                            pv,
                            kvxT_sb[:, cc * k + kn * 128:
                                    cc * k + kn * 128 + 128],
                            wv_sb[:, cc * HW:cc * HW + HW],
                            start=(cc == 0),
                            stop=(cc == ncc - 1),
                        )
                    nc.vector.tensor_copy(
                        vA4[:, :, kn, 0:CH],
                        pv.rearrange("p (h c) -> p h c", h=HPC),
                    )
                # ones columns of v_aug (softmax denominator trick)
                nc.vector.memset(ones_st, 1.0)
                for h in range(HPC):
                    nc.vector.tensor_copy(
                        vA4[:, h, :, CH:VW],
                        ones_st[:, 0:nkt].rearrange("p (n c) -> p n c", c=1),
                    )

                # gT: sigmoid(x@wg + bg), per head (partition base 0)
                for qn in range(nqc):
                    for h in range(HPC):
                        pg = ph0ps.tile([CH, 512], f32, tag="projg", bufs=2,
                                        name=f"pg{qn}_{h}")
                        for cc in range(ncc):
                            nc.tensor.matmul(
                                pg,
                                wg_sb[:, cc * HW + h * CH:
                                      cc * HW + h * CH + CH],
                                qxT_sb[:, cc * q + qn * 512:
                                       cc * q + qn * 512 + 512],
                                start=(cc == 0),
                                stop=(cc == ncc - 1),
                            )
                        nc.scalar.activation(
                            gT[:, h * q + qn * 512:h * q + qn * 512 + 512],
                            pg,
                            AF.Sigmoid,
                            bias=bg_sb[:, h:h + 1],
                        )

            # ---------------- main loop -----------------------------------
            with tc.tile_pool(name="mainps", bufs=1, space="PSUM") as mps:
                o_aug = {}       # (qc, h) -> [33, 512] PSUM accumulator
                attn_map = {}    # qc -> per-head (attnA, attnB) tile pairs
                pend = []        # pending AV units (qc, h, kc)
                KSPLIT = nkt - 4  # k tiles >= KSPLIT cross into the next
                #                   iteration -> double-buffered tail tile

                def attn_ap(uqc, h, kc):
                    a, bt = attn_map[uqc][h]
                    if kc < KSPLIT:
                        return a[:, kc * 512:kc * 512 + 512]
                    return bt[:, (kc - KSPLIT) * 512:(kc - KSPLIT) * 512 + 512]

                def emit_av(uqc, h, kc):
                    if (uqc, h) not in o_aug:
                        o_aug[(uqc, h)] = mps.tile(
                            [VW, 512], f32, tag=f"av{h}", bufs=1,
                            name=f"oaug{uqc}_{h}")
                    nc.tensor.matmul(
                        o_aug[(uqc, h)],
                        vA4[:, h, kc, :],
                        attn_ap(uqc, h, kc),
                        start=(kc == 0),
                        stop=(kc == nkt - 1),
                    )

                def emit_gating(gqc):
                    # reciprocal of the denominator row; o_aug copied out to
                    # SBUF fp16 (frees PSUM early); broadcast 1/den over 32
                    # partitions on Pool; gate+normalize
                    o_sb = smallp.tile([VW, HPC * 512], f16, tag="osb",
                                       name=f"osb{gqc}")
                    recip = smallp.tile([1, HPC * 512], f32, tag="recip",
                                        name=f"recip{gqc}")
                    for h in range(HPC):
                        nc.vector.reciprocal(
                            recip[:, h * 512:h * 512 + 512],
                            o_aug[(gqc, h)][CH:CH + 1, :],
                        )
                        # GPSIMD cannot read PSUM -> DVE does this copy
                        nc.vector.tensor_copy(
                            o_sb[:, h * 512:h * 512 + 512], o_aug[(gqc, h)]
                        )
                        del o_aug[(gqc, h)]
                    r_bc = smallp.tile([CH, HPC * 512], f32, tag="rbc",
                                       name=f"rbc{gqc}")
                    nc.gpsimd.partition_broadcast(r_bc, recip)
                    gtmp = smallp.tile([CH, HPC * 512], f32, tag="gtmp",
                                       name=f"gtmp{gqc}")
                    for h in range(HPC):
                        nc.gpsimd.tensor_mul(
                            gtmp[:, h * 512:h * 512 + 512],
                            gT[:, h * q + gqc * 512:h * q + gqc * 512 + 512],
                            r_bc[:, h * 512:h * 512 + 512],
                        )
                    for h in range(HPC):
                        # all-SBUF multiply -> Pool (keeps DVE for the
                        # PSUM-reading work it alone can do)
                        nc.gpsimd.tensor_mul(
                            ogT[:, h * q + gqc * 512:h * q + gqc * 512 + 512],
                            gtmp[:, h * 512:h * 512 + 512],
                            o_sb[0:CH, h * 512:h * 512 + 512],
                        )

                def emit_proj(pqc):
                    # output projection for block pqc; rides the S ring so
                    # PSUM stays within 8 banks
                    op = mps.tile([128, 1024], f32, tag="S", bufs=3,
                                  name=f"op{pqc}")
                    for s in range(4):
                        qs = pqc * 4 + s
                        for h in range(HPC):
                            nc.tensor.matmul(
                                op[:, s * 256:s * 256 + 256],
                                ogT[:, h * q + qs * 128:h * q + qs * 128 + 128],
                                wo_sb[:, h * C:h * C + C],
                                start=(h == 0),
                                stop=(h == HPC - 1),
                            )
                    ob = obp.tile([128, 1024], f32, tag="ob", name=f"ob{pqc}")
                    nc.vector.tensor_copy(ob, op)
                    nc.sync.dma_start(
                        out=out_d[pqc * 512:pqc * 512 + 512, :].rearrange(
                            "(n p) c -> p n c", p=128
                        ),
                        in_=ob.rearrange("p (n c) -> p n c", c=C),
                    )

                def drain_av(cur_qc, kc2, limit=6, lag=2):
                    ready = [u for u in pend if u[0] < cur_qc] + [
                        u for u in pend
                        if u[0] == cur_qc and u[2] < kc2 * 2 - (lag - 1) * 2
                    ]
                    for u in ready[:limit]:
                        pend.remove(u)
                        emit_av(*u)

                def emit_tail(gqc):
                    # last block: post-processing in 128-col slices pipelined
                    # across DVE/Pool/PE/DMA so the serial chain is short
                    o_sb = smallp.tile([VW, HPC * 512], f16, tag="osb",
                                       name=f"osbT{gqc}")
                    op = mps.tile([128, 1024], f32, tag="S", bufs=3,
                                  name=f"opT{gqc}")
                    for sl in range(4):
                        cs = slice(sl * 128, sl * 128 + 128)
                        qs = gqc * 4 + sl
                        recip = smallp.tile([1, HPC * 128], f32, tag="recT",
                                            bufs=4, name=f"recT{sl}")
                        for h in range(HPC):
                            nc.vector.reciprocal(
                                recip[:, h * 128:h * 128 + 128],
                                o_aug[(gqc, h)][CH:CH + 1, cs],
                            )
                            nc.vector.tensor_copy(
                                o_sb[:, h * 512 + sl * 128:
                                     h * 512 + sl * 128 + 128],
                                o_aug[(gqc, h)][:, cs],
                            )
                        r_bc = smallp.tile([CH, HPC * 128], f32, tag="rbcT",
                                           bufs=4, name=f"rbcT{sl}")
                        nc.gpsimd.partition_broadcast(r_bc, recip)
                        gtmp = smallp.tile([CH, HPC * 128], f32, tag="gtT",
                                           bufs=4, name=f"gtT{sl}")
                        for h in range(HPC):
                            nc.gpsimd.tensor_mul(
                                gtmp[:, h * 128:h * 128 + 128],
                                gT[:, h * q + qs * 128:h * q + qs * 128 + 128],
                                r_bc[:, h * 128:h * 128 + 128],
                            )
                        for h in range(HPC):
                            nc.vector.tensor_mul(
                                ogT[:, h * q + qs * 128:h * q + qs * 128 + 128],
                                gtmp[:, h * 128:h * 128 + 128],
                                o_sb[0:CH, h * 512 + sl * 128:
                                     h * 512 + sl * 128 + 128],
                            )
                        for h in range(HPC):
                            nc.tensor.matmul(
                                op[:, sl * 256:sl * 256 + 256],
                                ogT[:, h * q + qs * 128:h * q + qs * 128 + 128],
                                wo_sb[:, h * C:h * C + C],
                                start=(h == 0),
                                stop=(h == HPC - 1),
                            )
                        ob = obp.tile([128, 256], f32, tag="obT", bufs=4,
                                      name=f"obT{sl}")
                        nc.vector.tensor_copy(ob, op[:, sl * 256:sl * 256 + 256])
                        nc.sync.dma_start(
                            out=out_d[qs * 128:qs * 128 + 128, :],
                            in_=ob,
                        )
                    for h in range(HPC):
                        del o_aug[(gqc, h)]

                for qc in range(nqc):
                    if qc + 1 < nqc:
                        eb_tiles[qc + 1] = emit_eb_dma(qc + 1)
                    ebt = eb_tiles.pop(qc)
                    attn_map[qc] = [
                        (attp.tile([128, KSPLIT * 512], f16, tag=f"attnA{h}",
                                   bufs=1, name=f"attnA{h}_{qc}"),
                         attp.tile([128, (nkt - KSPLIT) * 512], f16,
                                   tag=f"attnB{h}", bufs=2,
                                   name=f"attnB{h}_{qc}"))
                        for h in range(HPC)
                    ]

                    for kc2 in range(nkc2):
                        for h in range(HPC):
                            S = mps.tile([128, 1024], f32, tag="S", bufs=3,
                                         name=f"S{qc}_{kc2}_{h}")
                            for t in range(2):
                                kc = kc2 * 2 + t
                                nc.tensor.matmul(
                                    S[:, t * 512:t * 512 + 512],
                                    kT[h * CH:h * CH + CH,
                                       kc * 128:kc * 128 + 128],
                                    qT[h * CH:h * CH + CH,
                                       qc * 512:qc * 512 + 512],
                                )
                            ap_t = app.tile([128, 1024], f16, tag="ap",
                                            name=f"ap{qc}_{kc2}_{h}")
                            nc.scalar.activation(ap_t, S, AF.Exp)
                            if kc2 * 2 < KSPLIT:
                                mdst = attn_map[qc][h][0][
                                    :, kc2 * 1024:kc2 * 1024 + 1024]
                            else:
                                off = kc2 * 2 - KSPLIT
                                mdst = attn_map[qc][h][1][
                                    :, off * 512:off * 512 + 1024]
                            nc.vector.tensor_mul(
                                mdst,
                                ap_t,
                                ebt[h][:, kc2 * 1024:kc2 * 1024 + 1024],
                            )
                            for t in range(2):
                                pend.append((qc, h, kc2 * 2 + t))
                        # gating for the previous block goes BEFORE this
                        # slot's AV drain so the o_aug ring (bufs=1) sees its
                        # reads emitted before the next block's first write
                        if qc > 0 and kc2 == 2:
                            emit_gating(qc - 1)
                        if qc == nqc - 1:
                            drain_av(qc, kc2, limit=8, lag=1)
                        else:
                            drain_av(qc, kc2)
                        if qc > 0 and kc2 == 5:
                            emit_proj(qc - 1)
                            del attn_map[qc - 1]

                # drain: AV tail for the last block, then its sliced epilogue
                last = nqc - 1
                for u in list(pend):
                    pend.remove(u)
                    emit_av(*u)
                emit_tail(last)

    nc.compile()
    return nc


def _shard_inputs(q_x, kv_x, mask_bias, triangle_bias, w_q, w_k, w_v, w_g,
                  b_g, w_o, b_o):
    """Build the 8 per-core input maps (host-side layout + precompute)."""
    f16 = np.float16
    inv = 1.0 / math.sqrt(CH)
    in_maps = []
    for core in range(N_CORES):
        b = core // GROUPS
        g = core % GROUPS
        h0 = g * HPC
        cs = slice(h0 * CH, (h0 + HPC) * CH)
        m = {
            "qxT": np.ascontiguousarray(q_x[b].T).astype(f16),
            "kvxT": np.ascontiguousarray(kv_x[b].T).astype(f16),
            "wq": (w_q[:, cs] * inv).astype(f16),
            "wk": w_k[:, cs].astype(f16),
            "wv": w_v[:, cs].astype(f16),
            "wg": w_g[:, cs].astype(f16),
            "bg": np.ascontiguousarray(
                b_g[cs].reshape(HPC, CH).T).astype(np.float32),
            "wo": np.ascontiguousarray(
                w_o[cs, :].reshape(HPC, CH, C).transpose(1, 0, 2)
            ).reshape(CH, HPC * C).astype(f16),
        }
        mk = mask_bias[b, 0, 0]  # [K]
        for h in range(HPC):
            eb = np.exp(triangle_bias[b, h0 + h] + mk[None, :])
            m[f"eb{h}"] = np.ascontiguousarray(eb.T).astype(f16)
        in_maps.append(m)
    return in_maps


def kernel(**inputs):
    from concourse import bass_utils

    inputs = {k_: np.asarray(v, dtype=np.float32) for k_, v in inputs.items()}
    if "nc" not in _cache:
        _cache["nc"] = _build_nc()
    nc = _cache["nc"]

    in_maps = _shard_inputs(**inputs)
    res = bass_utils.run_bass_kernel_spmd(nc, in_maps,
                                          core_ids=list(range(N_CORES)))

    out = np.zeros((B, Q, C), np.float32)
    for core in range(N_CORES):
        out[core // GROUPS] += res.results[core]["out_p"]
    out += inputs["b_o"][None, None, :]
    return out
